# revision 23
# baseline (speedup 1.0000x reference)
"""Trainium2 Bass kernel for nn_PredCells (3-layer predictive-coding LSTM stack).

Strategy
--------
The recurrence is strictly sequential in t; batch=1, so the only useful
parallelism is tensor parallelism within each step.  We restructure the
math so each timestep needs exactly ONE 8-core AllGather:

* The f-gate is dead (c0 = 0), so each LSTM needs only [i; o; g] rows.
* All inter-layer linear chains are folded (on the host, in float64) into
  per-state product matrices:
      z1(t) = A11 s1(t-1) + A12 s2(t-2) + B1 x_t + c1
      z2(t) = A21 s1(t)   + A22 s2(t-1) + A23 s3(t-2) + c2
      z3(t) = A32 s2(t)   + A33 s3(t-1) + c3
      s_l   = sigmoid(o) * tanh(sigmoid(i) * tanh(g))
  which admits a wavefront schedule: tick k computes s1(k), s2(k-1),
  s3(k-2) — all inputs come from tick k-1 → one batched AllGather per
  tick of the three fresh 128-element state shards.
* Weights are sharded 8-way by output rows (gate-blocked) and kept
  resident in SBUF.  The A matrices are stored fp8e4 scaled by 2^b and
  the state stationary is fp8e4 scaled by 2^a, so the z matvecs run in
  DoubleRow perf mode (2 contraction rows/partition, 2x column rate).
  The descale 2^-(a+b) rides the gate ACTIVATE's scale; the bias
  vectors are pre-scaled by 2^(a+b) and injected into PSUM via tiny
  K=1 matmuls.  V mats are bf16 scaled 2^-a so the loss reconstructions
  come out unscaled.  (Validated on host: total rel err ~3e-5.)
* The gathered shards land in DRAM as [8, 384]; they are loaded
  contiguously and transposed to the [128, 24] stationary layout with
  three PE transpose matmuls (a 2-byte scatter DMA here costs ~4us).
* Loss terms are computed off the critical path from per-core row
  shards via ACT Abs accum_out.

The kernel returns per-core partial sums of the three loss terms; the
host combines them with lambda and sums across cores.
"""

import numpy as np
import ml_dtypes

import concourse.mybir as mybir
import concourse.tile as tile
from concourse import bacc
from concourse.bass_utils import run_bass_kernel_spmd

H = 1024
C = 56
NC = 8
P = 128
NCH = H // P  # 8 K-chunks of 128 per H-sized contraction
A_EXP = 6     # state scale 2^a

F32 = mybir.dt.float32
BF16 = mybir.dt.bfloat16
F8 = mybir.dt.float8e4
NP_BF16 = ml_dtypes.bfloat16
NP_F8 = ml_dtypes.float8_e4m3

_NC_CACHE = {}


# ----------------------------------------------------------------------------
# Host-side weight preparation
# ----------------------------------------------------------------------------

def _gate_rows(Wih):
    # reorder LSTM gate blocks to [i; o; g] so sigmoid covers one
    # contiguous 256-wide slice per region on device
    return np.concatenate([Wih[0:H], Wih[3 * H:4 * H], Wih[2 * H:3 * H]], axis=0)


def _prep_host(inputs):
    """Product-form parameters (float64) + per-core input maps."""
    g = lambda k: np.asarray(inputs[k], np.float64)
    W0, W0b = g("W0_w"), g("W0_b")
    W1, W1b = g("W1_w"), g("W1_b")
    W2, W2b = g("W2_w"), g("W2_b")
    V1, V1b = g("V1_w"), g("V1_b")
    V2, V2b = g("V2_w"), g("V2_b")
    V3, V3b = g("V3_w"), g("V3_b")
    Wih1, b1 = _gate_rows(g("Wih1")), _gate_rows(g("b1")[:, None])[:, 0]
    Wih2, b2 = _gate_rows(g("Wih2")), _gate_rows(g("b2")[:, None])[:, 0]
    Wih3, b3 = _gate_rows(g("Wih3")), _gate_rows(g("b3")[:, None])[:, 0]
    W1L, W1R = Wih1[:, :H], Wih1[:, H:]
    W2L, W2R = Wih2[:, :H], Wih2[:, H:]

    A = {
        "A11": W1R - W1L @ W0 @ V1,
        "A12": -W1R @ V2,
        "A21": W2L @ W1,
        "A22": W2R - W2L @ W1 @ V2,
        "A23": -W2R @ V3,
        "A32": Wih3 @ W2,
        "A33": -Wih3 @ W2 @ V3,
    }
    B1 = W1L @ W0  # [3H, C]

    absmax = max(np.abs(M).max() for M in A.values())
    b_exp = int(np.floor(np.log2(300.0 / absmax)))
    SA, SB = float(2.0 ** A_EXP), float(2.0 ** b_exp)

    c1_0 = b1 + W1L @ W0b
    c1_1 = c1_0 - W1L @ (W0 @ V1b)
    c1_2 = c1_1 - W1R @ V2b
    c2_0 = b2 + W2L @ W1b
    c2_1 = c2_0 - W2L @ (W1 @ V2b)
    c2_2 = c2_1 - W2R @ V3b
    c3_0 = b3 + Wih3 @ W2b
    c3_1 = c3_0 - Wih3 @ (W2 @ V3b)
    cz = [[c1_0, c1_1, c1_2], [c2_0, c2_1, c2_2], [c3_0, c3_1, c3_1]]

    x = np.asarray(inputs["input_sentence"], np.float64)  # [T, C]
    Tn = x.shape[0]

    def shard_rows(M, c):
        idx = np.r_[c * P:(c + 1) * P, H + c * P:H + (c + 1) * P,
                    2 * H + c * P:2 * H + (c + 1) * P]
        return M[idx]

    def chunked_T(Mc):
        """[rows, K] -> transpose -> chunk K into [P, nch*rows] (chunk-major)."""
        MT = np.ascontiguousarray(Mc.T)  # [K, rows]
        K = MT.shape[0]
        nch = K // P
        return np.concatenate([MT[i * P:(i + 1) * P] for i in range(nch)], axis=1)

    in_maps = []
    for c in range(NC):
        m = {}
        for name, M in A.items():
            m["w_" + name] = chunked_T(shard_rows(M * SB, c)).astype(NP_F8)  # [128, 8*384]
        m["w_B1"] = np.ascontiguousarray(
            shard_rows(B1 * SA * SB, c).T).astype(NP_BF16)  # [56, 384]
        # V mats (loss recons), scaled 2^-a so rp = V s_q is unscaled
        V1c = V1[7 * c:7 * (c + 1)] / SA       # [7, H]
        V2c = V2[P * c:P * (c + 1)] / SA       # [128, H]
        V3c = V3[P * c:P * (c + 1)] / SA
        m["w_V1"] = chunked_T(V1c).astype(NP_BF16)   # [128, 8*7]
        m["w_V2"] = chunked_T(V2c).astype(NP_BF16)   # [128, 8*128]
        m["w_V3"] = chunked_T(V3c).astype(NP_BF16)
        # x: stationary [C, T] bf16 (dynamics) + per-core rows [1, 7T] f32 (loss)
        m["x_stat"] = np.ascontiguousarray(x.T).astype(NP_BF16)        # [56, T]
        m["x_rows"] = np.ascontiguousarray(
            x[:, 7 * c:7 * (c + 1)].reshape(1, -1)).astype(np.float32)  # [1, 7T]
        # bias rows: 4 variants (tick 0,1,2,>=3) x 3 regions, scaled 2^(a+b)
        bias = np.zeros((1, 4 * 1152), np.float64)
        for v in range(4):
            for z in range(3):
                t_z = v - z
                if t_z < 0:
                    continue
                vec = cz[z][min(t_z, 2)]
                for gi in range(3):
                    bias[0, v * 1152 + z * 384 + gi * P:
                         v * 1152 + z * 384 + (gi + 1) * P] = \
                        vec[gi * H + c * P:gi * H + (c + 1) * P]
        m["biases"] = (bias * SA * SB).astype(NP_BF16)
        m["V1b_row"] = np.ascontiguousarray(V1b[None, 7 * c:7 * (c + 1)]).astype(np.float32)
        m["V2b_row"] = np.ascontiguousarray(V2b[None, P * c:P * (c + 1)]).astype(np.float32)
        m["V3b_row"] = np.ascontiguousarray(V3b[None, P * c:P * (c + 1)]).astype(np.float32)
        m["ident8"] = np.eye(8, dtype=NP_BF16)
        m["ones11"] = np.ones((1, 1), NP_BF16)
        in_maps.append(m)

    lam = 1e-4 if int(np.asarray(inputs["iternumber"])) <= 1000 else 1e-2
    return in_maps, lam, Tn, b_exp


# ----------------------------------------------------------------------------
# Device kernel
# ----------------------------------------------------------------------------

def _build_nc(Tn, b_exp, dump=False):
    nc = bacc.Bacc("TRN2", target_bir_lowering=False, debug=False, num_devices=NC)

    DESCALE = float(2.0 ** (-(A_EXP + b_exp)))
    SA = float(2.0 ** A_EXP)

    ext = {}
    shapes = {
        "w_A11": ([P, NCH * 384], F8), "w_A12": ([P, NCH * 384], F8),
        "w_A21": ([P, NCH * 384], F8), "w_A22": ([P, NCH * 384], F8),
        "w_A23": ([P, NCH * 384], F8), "w_A32": ([P, NCH * 384], F8),
        "w_A33": ([P, NCH * 384], F8),
        "w_B1": ([C, 384], BF16),
        "w_V1": ([P, NCH * 7], BF16),
        "w_V2": ([P, NCH * P], BF16),
        "w_V3": ([P, NCH * P], BF16),
        "x_stat": ([C, Tn], BF16),
        "x_rows": ([1, 7 * Tn], F32),
        "biases": ([1, 4 * 1152], BF16),
        "V1b_row": ([1, 7], F32),
        "V2b_row": ([1, P], F32),
        "V3b_row": ([1, P], F32),
        "ident8": ([8, 8], BF16),
        "ones11": ([1, 1], BF16),
    }
    for name, (shape, dt) in shapes.items():
        ext[name] = nc.dram_tensor(name, shape, dt, kind="ExternalInput")
    out_terms = nc.dram_tensor("terms", [1, 3], F32, kind="ExternalOutput")
    out_sdump = nc.dram_tensor("sdump", [1, 384 * Tn], BF16, kind="ExternalOutput") if dump else None

    NT = Tn - 1  # dynamics ticks 0..NT-1; loss tail tick NT
    Sig = mybir.ActivationFunctionType.Sigmoid
    Tanh = mybir.ActivationFunctionType.Tanh
    Abs = mybir.ActivationFunctionType.Abs
    Copy = mybir.ActivationFunctionType.Copy
    DR = mybir.MatmulPerfMode.DoubleRow

    with tile.TileContext(nc) as tc:
        with (
            tc.tile_pool(name="w", bufs=1) as wp,
            tc.tile_pool(name="s", bufs=3) as sp,
            tc.tile_pool(name="acc", bufs=1) as ap,
            tc.tile_pool(name="zp", bufs=2, space="PSUM") as zpp,
            tc.tile_pool(name="rp", bufs=1, space="PSUM") as rpp,
            tc.tile_pool(name="tp", bufs=1, space="PSUM") as tpp,
            tc.tile_pool(name="dram", bufs=1, space="DRAM") as dp,
        ):
            # ---- load weights/constants to SBUF once ----
            W = {}
            for name, (shape, dt) in shapes.items():
                t = wp.tile(shape, dt, tag=name, name=name)
                nc.sync.dma_start(t[:], ext[name][:])
                W[name] = t

            acc = [ap.tile([1, Tn], F32, tag=f"acc{j}", name=f"acc{j}") for j in range(3)]
            for a in acc:
                nc.vector.memset(a[:], 0.0)

            def a_pair(name, p):
                mv = W["w_" + name][:, (2 * p) * 384:(2 * p + 2) * 384]
                return mv.rearrange("k (two n) -> k two n", two=2)

            def st_pair(stat, j, p):
                # [128, 2, 1] stationary: even chunk 2p (half 0), odd 2p+1 (half 1)
                sr = stat.rearrange("k (two m) -> k two m", two=2)
                return sr[:, :, 4 * j + p:4 * j + p + 1]

            def st_one(stat, j, ch):
                # [128, 1] single-chunk stationary for plain matmuls
                col = (ch % 2) * 16 + 4 * j + ch // 2
                return stat[:, col:col + 1]

            s_hist = {}       # tick -> s_all [1, 384] bf16 (own shards, loss path)
            bo_hist = {}      # tick -> AG output dram tile [8, 384] bf16

            for k in range(NT + 1):
                dyn = k < NT
                nz = min(k + 1, 3) if dyn else 0
                n_stat = min(k, 3)   # states present in AG(k-1)

                # ---- stat build: bo load (3 queues) -> PE transposes -> copy
                stat = None
                if n_stat >= 1:
                    bo_sb = sp.tile([8, 384], BF16, tag="bo_sb", name="bo_sb")
                    bo = bo_hist[k - 1]
                    nc.sync.dma_start(bo_sb[:, 0:192], bo[:, 0:192])
                    nc.scalar.dma_start(bo_sb[:, 192:384], bo[:, 192:384])
                    stat = sp.tile([P, 32], F8, tag="stat", name="stat")
                    stat_r = stat.rearrange("k (two m) -> k two m", two=2)
                    tp = tpp.tile([P, 24], BF16, tag="tp", name="tp")
                    for j in range(n_stat):
                        nc.tensor.matmul(
                            tp[:, 8 * j:8 * j + 8], bo_sb[:, P * j:P * (j + 1)],
                            W["ident8"][:],
                            start=True, stop=True,
                            is_transpose=True, skip_group_check=True,
                        )
                    tp_r = tp.rearrange("k (j c2 two) -> k two (j c2)",
                                        j=3, c2=4, two=2)
                    nc.vector.tensor_scalar_mul(stat_r[:, :, 0:12], tp_r, SA)

                # ---- PSUM tiles ----
                if dyn:
                    zp = zpp.tile([1, 1536], F32, tag="zp", name="zp")
                    bv = min(k, 3)
                rp = rpp.tile([1, 512], F32, tag="rp", name="rp")

                def a_mms(outp, name, j, stop=False):
                    for p in range(4):
                        nc.tensor.matmul(
                            outp, st_pair(stat, j, p), a_pair(name, p),
                            start=False, stop=(stop and p == 3),
                            perf_mode=DR, skip_group_check=True,
                        )

                # ---- z matvecs, region-sequential accumulation groups ----
                if dyn:
                    for z in range(nz):
                        outp = zp[0:1, z * 512:z * 512 + 384]
                        nc.tensor.matmul(
                            outp, W["ones11"][:],
                            W["biases"][0:1, bv * 1152 + z * 384:bv * 1152 + (z + 1) * 384],
                            start=True, stop=False, skip_group_check=True,
                        )
                        if z == 0:
                            nc.tensor.matmul(
                                outp, W["x_stat"][:, k:k + 1], W["w_B1"][:],
                                start=False, stop=(k == 0), skip_group_check=True,
                            )
                            if k >= 1:
                                a_mms(outp, "A11", 0, stop=(k == 1))
                            if k >= 2:
                                a_mms(outp, "A12", 1, stop=True)
                        elif z == 1:
                            a_mms(outp, "A21", 0, stop=(k == 1))
                            if k >= 2:
                                a_mms(outp, "A22", 1, stop=(k == 2))
                            if k >= 3:
                                a_mms(outp, "A23", 2, stop=True)
                        else:
                            a_mms(outp, "A32", 1, stop=(k == 2))
                            if k >= 3:
                                a_mms(outp, "A33", 2, stop=True)

                # ---- r matvecs after z so gates fire asap ----
                r_specs = []
                if k >= 1:
                    r_specs.append(("w_V1", 7, 0, 0))
                if k >= 2:
                    r_specs.append(("w_V2", P, 1, P))
                if k >= 3:
                    r_specs.append(("w_V3", P, 2, 2 * P))
                for wname, mw, sj, ro in r_specs:
                    for ch in range(NCH):
                        nc.tensor.matmul(
                            rp[0:1, ro:ro + mw],
                            st_one(stat, sj, ch),
                            W[wname][:, ch * mw:(ch + 1) * mw],
                            start=(ch == 0), stop=(ch == NCH - 1),
                            skip_group_check=True,
                        )

                # ---- gates straight from PSUM, batched across regions ----
                if dyn:
                    # gates in two groups: (z1,z2) overlap z3's matmuls; z3's
                    # short chain is all that remains after the last matmul
                    zpr = zp.rearrange("p (z c) -> p z c", z=3)
                    s_all = sp.tile([1, 384], BF16, tag="sall", name="s_all")
                    ga_sio = sp.tile([1, 512], F32, tag="gsio", name="ga_sio")
                    nc.scalar.activation(ga_sio[:], zpr[:, 0:2, 0:256], Sig,
                                         scale=DESCALE)
                    ga_gg = sp.tile([1, 256], F32, tag="ggg", name="ga_gg")
                    nc.scalar.activation(ga_gg[:], zpr[:, 0:2, 256:384], Tanh,
                                         scale=DESCALE)
                    ga_r = ga_sio.rearrange("p (z h c) -> p z h c", z=2, h=2)
                    ga_t1 = sp.tile([1, 256], F32, tag="gt1", name="ga_t1")
                    nc.vector.tensor_mul(ga_t1[:], ga_r[:, :, 0, :], ga_gg[:])
                    ga_t2 = sp.tile([1, 256], F32, tag="gt2", name="ga_t2")
                    nc.scalar.activation(ga_t2[:], ga_t1[:], Tanh)
                    nc.vector.tensor_mul(s_all[0:1, 0:256], ga_r[:, :, 1, :],
                                         ga_t2[:])
                    gb_sio = sp.tile([1, 256], F32, tag="bsio", name="gb_sio")
                    nc.scalar.activation(gb_sio[:], zp[0:1, 1024:1280], Sig,
                                         scale=DESCALE)
                    gb_gg = sp.tile([1, P], F32, tag="bgg", name="gb_gg")
                    nc.scalar.activation(gb_gg[:], zp[0:1, 1280:1408], Tanh,
                                         scale=DESCALE)
                    gb_t1 = sp.tile([1, P], F32, tag="bt1", name="gb_t1")
                    nc.vector.tensor_mul(gb_t1[:], gb_sio[0:1, 0:P], gb_gg[:])
                    gb_t2 = sp.tile([1, P], F32, tag="bt2", name="gb_t2")
                    nc.scalar.activation(gb_t2[:], gb_t1[:], Tanh)
                    nc.vector.tensor_mul(s_all[0:1, 256:384], gb_sio[0:1, P:256],
                                         gb_t2[:])
                    s_hist[k] = s_all
                    if dump:
                        nc.scalar.dma_start(out_sdump[:, 384 * k:384 * (k + 1)],
                                            s_all[:])

                    bi = dp.tile([1, 384], BF16, tag=f"bi{k}", name=f"bi{k}")
                    nc.sync.dma_start(bi[:], s_all[:])
                    bo_new = dp.tile([NC, 384], BF16, tag=f"bo{k}", name=f"bo{k}",
                                     addr_space="Shared")
                    nc.gpsimd.collective_compute(
                        "AllGather", mybir.AluOpType.bypass,
                        replica_groups=[list(range(NC))],
                        ins=[bi.opt()], outs=[bo_new.opt()],
                    )
                    bo_hist[k] = bo_new

                # ---- loss terms (rows; accumulated via ACT Abs accum_out) ----
                junk = sp.tile([1, P], F32, tag="junk", name="junk")
                d = sp.tile([1, P], F32, tag="d", name="d")
                if k == 0:
                    nc.scalar.activation(junk[0:1, 0:7], W["x_rows"][0:1, 0:7], Abs,
                                         accum_out=acc[0][0:1, 0:1])
                else:
                    sprev = s_hist[k - 1]
                    nc.vector.tensor_sub(d[0:1, 0:7], W["x_rows"][0:1, 7 * k:7 * k + 7],
                                         rp[0:1, 0:7])
                    nc.vector.tensor_sub(d[0:1, 0:7], d[0:1, 0:7], W["V1b_row"][0:1, :])
                    nc.scalar.activation(junk[0:1, 0:7], d[0:1, 0:7], Abs,
                                         accum_out=acc[0][0:1, k:k + 1])
                    if k == 1:
                        nc.scalar.activation(junk[0:1, :], sprev[0:1, 0:P], Abs,
                                             accum_out=acc[1][0:1, 1:2])
                    else:
                        d1 = sp.tile([1, P], F32, tag="d1", name="d1")
                        nc.vector.tensor_sub(d1[:], sprev[0:1, 0:P], rp[0:1, P:2 * P])
                        nc.vector.tensor_sub(d1[:], d1[:], W["V2b_row"][0:1, :])
                        nc.scalar.activation(junk[0:1, :], d1[:], Abs,
                                             accum_out=acc[1][0:1, k:k + 1])
                        if k == 2:
                            nc.scalar.activation(junk[0:1, :], sprev[0:1, P:2 * P], Abs,
                                                 accum_out=acc[2][0:1, 2:3])
                        else:
                            d2 = sp.tile([1, P], F32, tag="d2", name="d2")
                            nc.vector.tensor_sub(d2[:], sprev[0:1, P:2 * P],
                                                 rp[0:1, 2 * P:3 * P])
                            nc.vector.tensor_sub(d2[:], d2[:], W["V3b_row"][0:1, :])
                            nc.scalar.activation(junk[0:1, :], d2[:], Abs,
                                                 accum_out=acc[2][0:1, k:k + 1])

            # ---- final reduction ----
            finrow = ap.tile([1, 3], F32, tag="finrow", name="finrow")
            for j in range(3):
                nc.vector.tensor_reduce(finrow[0:1, j:j + 1], acc[j][:],
                                        mybir.AxisListType.X, mybir.AluOpType.add)
            nc.sync.dma_start(out_terms[:], finrow[:])

    nc.compile()
    return nc


def _get_nc(Tn, b_exp, dump=False):
    key = (Tn, b_exp, dump)
    if key not in _NC_CACHE:
        _NC_CACHE[key] = _build_nc(Tn, b_exp, dump)
    return _NC_CACHE[key]


def _run(inputs, trace=False, dump=False):
    in_maps, lam, Tn, b_exp = _prep_host(inputs)
    nc = _get_nc(Tn, b_exp, dump)
    res = run_bass_kernel_spmd(nc, in_maps, core_ids=list(range(NC)), trace=trace)
    terms = np.zeros(3, np.float64)
    for r in res.results:
        terms += np.asarray(r["terms"][0], np.float64)
    loss = terms[0] + lam * terms[1] + lam * lam * terms[2]
    return np.float32(loss), res


def kernel(**inputs):
    loss, _ = _run(inputs)
    return loss
